# revision 11
# baseline (speedup 1.0000x reference)
"""Trainium2 Bass kernel for nn_CoordinateDescent (B=2, M=N=2048, R=16).

Math: the coordinate-descent residual e never needs materializing. With
G = v^T v and c = x @ v, the per-rank recurrence collapses to a 16x16
triangular solve per row, computed via the nilpotent expansion
(I+Z)^-1 = (I-Z)(I+Z^2)(I+Z^4)(I+Z^8) -- log-depth small matmuls.

Sharding: 8 cores = batch (2) x N-shard (4). Every core computes the FULL
u_new for its batch from the full x^T (redundant 4x within a batch-group
but collective-free: measured ncfw AllGather here costs 60-80us dead time,
far more than the extra DMA). Phase 2 updates the core's own v N-shard.

Perf notes (v3): x^T is packed chunk-major in DRAM ([P, MC, KO, SHC]) so
every DMA descriptor is an 8KB contiguous run on both DRAM and SBUF
sides (1KB-run descriptors run at lower per-engine rates). xtf streams
in 4 chunks of 512 m-columns -- 512-wide matmul rhs amortizes the
~50ns/instruction PE overhead that 256-wide chunks double. The per-chunk
a-tile matmuls (ap1), bf16 cast, and phase-2 Gram partials all ride the
DMA-paced stream instead of the post-stream tail; xn follows xtf on the
same sync ring (FIFO keeps it from stealing stream bandwidth), quartered
so phase-2's c2 accumulation starts as soon as its slices land. The
phase-2 solve chain runs on vector/gpsimd/scalar under the xn stream.
Remote-DMA u-chunk exchange (which would cut the 4x x^T redundancy) was
prototyped and works functionally, but blocked waits on remotely-written
semaphores take 2-11ms in this runtime -- unusable.
"""

import os
import numpy as np
import ml_dtypes

import concourse.bass as bass
import concourse.mybir as mybir
import concourse.tile as tile
from concourse import bacc
from concourse.bass_utils import run_bass_kernel_spmd
from concourse.masks import make_identity

B, M, N, R = 2, 2048, 2048, 16
NCORES = 8
NS = 4            # N-shards per batch (cores per batch group)
SH = M // NS      # 512 (n-shard width)
P = 128
KO = M // P       # 16 k-tiles of 128
TS = SH // P      # 4 sub-tiles per shard
MC = 4            # m-chunks for the phase-1 cT stream
SHC = M // MC     # 512 m per chunk
XQ = 4            # xn stream quarters
EPS = 1e-8

F32 = mybir.dt.float32
BF16 = mybir.dt.bfloat16
ALU = mybir.AluOpType

_CACHE = {}


def _build_nc():
    nc = bacc.Bacc(
        "TRN2",
        target_bir_lowering=False,
        debug=False,
        num_devices=NCORES,
    )

    # x[b]^T chunk-major: [P, MC, KO, SHC]; per (p, chunk): 8KB contiguous
    xtf_d = nc.dram_tensor("xtf", [P, MC, KO, SHC], BF16, kind="ExternalInput")
    # x[b,:,nS] m-tiled: [P, KO, SH]; per p: 16KB contiguous
    xn_d = nc.dram_tensor("xn", [P, KO, SH], BF16, kind="ExternalInput")
    vf_d = nc.dram_tensor("vf", [P, KO, R], F32, kind="ExternalInput")     # v[b] tiled f32
    vb_d = nc.dram_tensor("vb", [P, KO, R], BF16, kind="ExternalInput")    # v[b] tiled bf16
    ut_d = nc.dram_tensor("ut", [R, M], BF16, kind="ExternalInput")        # full u[b]^T
    vt_d = nc.dram_tensor("vt", [R, SH], BF16, kind="ExternalInput")       # v[b,nS,:]^T
    ou_d = nc.dram_tensor("ou", [P, KO, R], F32, kind="ExternalOutput")    # full u_new
    ov_d = nc.dram_tensor("ov", [P, TS, R], F32, kind="ExternalOutput")    # v_new shard

    with tile.TileContext(nc, num_cores=NCORES) as tc:
        with (
            tc.tile_pool(name="big", bufs=1) as big,
            tc.tile_pool(name="cst", bufs=1) as cst,
            tc.tile_pool(name="ya", bufs=1) as ya,
            tc.tile_pool(name="gps", bufs=1, space="PSUM") as gps,
            tc.tile_pool(name="sps", bufs=2, space="PSUM") as sps,
            tc.tile_pool(name="bps", bufs=2, space="PSUM") as bps,
            tc.tile_pool(name="aps", bufs=1, space="PSUM") as aps,
        ):
            vf = cst.tile([P, KO, R], F32, tag="vf")
            vb = cst.tile([P, KO, R], BF16, tag="vb")
            utf = cst.tile([P, M], BF16, tag="utf")     # rows 16+ zero
            vt = cst.tile([P, SH], BF16, tag="vt")      # rows 16+ zero
            ident = cst.tile([P, R], F32, tag="ident")  # I16 in rows 0:16
            misc = cst.tile([P, 8], F32, tag="misc")    # d / rd columns
            anat = cst.tile([P, KO, R], F32, tag="anat")
            ab16 = cst.tile([P, KO, R], BF16, tag="ab16")
            vnat = cst.tile([P, TS, R], F32, tag="vnat")

            NSLOT = 18
            arena = cst.tile([P, 2 * NSLOT, R], F32, tag="arena")
            sun16 = cst.tile([P, 2, R], BF16, tag="sun16")
            wzb = cst.tile([P, 2, R], BF16, tag="wzb")
            yb1 = ya.tile([P, M], BF16, tag="yb1")      # rows 16+ zero
            yb2 = ya.tile([P, SH], BF16, tag="yb2")

            nc.any.memzero(utf[:])
            nc.any.memzero(vt[:])
            nc.any.memzero(ident[:])
            nc.any.memzero(arena[:])
            nc.any.memzero(sun16[:])
            nc.any.memzero(wzb[:])
            nc.any.memzero(yb1[:])
            nc.any.memzero(yb2[:])
            make_identity(nc, ident[0:R, 0:R], nomemset=True)

            # Small inputs lead the sync ring; the big streams follow FIFO.
            nc.sync.dma_start(vf[:], vf_d[:])
            nc.sync.dma_start(vb[:], vb_d[:])
            nc.sync.dma_start(utf[0:R, :], ut_d[:])
            nc.sync.dma_start(vt[0:R, :], vt_d[:])

            xtf = big.tile([P, MC, KO, SHC], BF16, tag="xtf")
            xn = big.tile([P, KO, SH], BF16, tag="xn")
            # ko-sliced chunk DMAs (8KB/partition runs) so the c-matmuls
            # start as soon as the first k-tiles land, not per 2MB chunk;
            # the kernel is PE-bound end to end, so an earlier PE start
            # shifts the whole pipeline left.
            for j in range(MC):
                kq = 4 if j == 0 else 8
                for q in range(KO // kq):
                    s = slice(q * kq, (q + 1) * kq)
                    nc.sync.dma_start(xtf[:, j, s, :], xtf_d[:, j, s, :])
            KQ = KO // XQ
            for q in range(XQ):
                s = slice(q * KQ, (q + 1) * KQ)
                nc.sync.dma_start(xn[:, s, :], xn_d[:, s, :])

            def slot(ph, i):
                return arena[:, ph * NSLOT + i, :]

            def slot16(ph, i):
                return arena[0:R, ph * NSLOT + i, :]

            def smm(out_slot16, lhsT_pad, rhs_pad):
                ps = sps.tile([R, R], F32, tag="sps")
                nc.tensor.matmul(ps[:], lhsT_pad, rhs_pad)
                nc.any.tensor_copy(out=out_slot16, in_=ps[:])

            I16 = ident[0:R, 0:R]

            def small_chain(ph, g_psum):
                G = slot16(ph, 0)
                nc.any.tensor_copy(out=G, in_=g_psum[:])
                d = misc[0:R, 4 * ph + 0 : 4 * ph + 1]
                rd = misc[0:R, 4 * ph + 1 : 4 * ph + 2]
                gd = slot16(ph, 1)
                nc.vector.tensor_tensor(gd, G, I16, ALU.mult)
                nc.vector.tensor_reduce(d, gd, axis=mybir.AxisListType.X, op=ALU.add)
                nc.vector.tensor_scalar_add(d, d, float(EPS))
                nc.vector.reciprocal(rd, d)
                nc.any.tensor_scalar_mul(gd, G, -1.0)
                slnf = slot16(ph, 15)
                nc.gpsimd.affine_select(
                    out=slnf, in_=gd, compare_op=ALU.is_ge, fill=0.0,
                    base=-1, pattern=[[-1, R]], channel_multiplier=1,
                )
                nc.any.tensor_copy(out=sun16[0:R, ph, :], in_=slnf)
                SL = slot16(ph, 2)
                nc.gpsimd.affine_select(
                    out=SL, in_=G, compare_op=ALU.is_ge, fill=0.0,
                    base=-1, pattern=[[-1, R]], channel_multiplier=1,
                )
                Z = slot16(ph, 3)
                nc.vector.tensor_scalar_mul(Z, SL, rd)
                smm(slot16(ph, 4), slot(ph, 3), ident[:, 0:R])  # zt1 = Z^T
                smm(slot16(ph, 5), slot(ph, 4), slot(ph, 3))   # z2
                smm(slot16(ph, 6), slot(ph, 3), slot(ph, 4))   # zt2
                smm(slot16(ph, 7), slot(ph, 6), slot(ph, 5))   # z4
                smm(slot16(ph, 8), slot(ph, 5), slot(ph, 6))   # zt4
                smm(slot16(ph, 9), slot(ph, 8), slot(ph, 7))   # z8
                nc.vector.tensor_tensor(slot16(ph, 10), I16, slot16(ph, 4), ALU.subtract)
                nc.vector.tensor_tensor(slot16(ph, 11), I16, slot16(ph, 5), ALU.add)
                nc.vector.tensor_tensor(slot16(ph, 12), I16, slot16(ph, 8), ALU.add)
                nc.vector.tensor_tensor(slot16(ph, 13), I16, slot16(ph, 9), ALU.add)
                smm(slot16(ph, 14), slot(ph, 11), slot(ph, 10))  # P1
                smm(slot16(ph, 15), slot(ph, 12), slot(ph, 13))  # o2T
                smm(slot16(ph, 16), slot(ph, 15), slot(ph, 14))  # WzT
                nc.any.tensor_copy(out=wzb[0:R, ph, :], in_=slot16(ph, 16))
                return rd

            # ================= phase 1: full u update =================
            gp = gps.tile([R, R], F32, tag="gps")
            for ko in range(KO):
                nc.tensor.matmul(
                    gp[:], vf[:, ko, :], vf[:, ko, :],
                    start=(ko == 0), stop=(ko == KO - 1),
                )
            TPC = KO // MC   # ap1/G2 tiles finished per chunk
            rd1 = None
            ap1 = aps.tile([P, KO * R], F32, tag="aps1")
            gp2 = gps.tile([R, R], F32, tag="gps2")
            for j in range(MC):
                ms = slice(j * SHC, (j + 1) * SHC)
                ct = bps.tile([R, SHC], F32, tag="bps")
                for ko in range(KO):
                    nc.tensor.matmul(
                        ct[:], vb[:, ko, :], xtf[:, j, ko, :],
                        start=(ko == 0), stop=False,
                    )
                if j == 0:
                    rd1 = small_chain(0, gp)   # hides in the DMA-paced stream
                nc.tensor.matmul(
                    ct[:], sun16[:, 0, :], utf[:, ms], start=False, stop=True
                )
                nc.vector.tensor_scalar(
                    out=yb1[0:R, ms], in0=ct[:], scalar1=float(EPS), scalar2=rd1,
                    op0=ALU.add, op1=ALU.mult,
                )
                # ap1 tiles, bf16 cast, and G2 partials for this chunk ride
                # the DMA-paced stream instead of the post-stream tail
                for t in range(j * TPC, (j + 1) * TPC):
                    nc.tensor.matmul(
                        ap1[:, t * R : (t + 1) * R],
                        yb1[:, t * P : (t + 1) * P], wzb[:, 0, :],
                    )
                nc.vector.tensor_copy(
                    out=ab16[:, j * TPC : (j + 1) * TPC, :].rearrange(
                        "p t r -> p (t r)"
                    ),
                    in_=ap1[:, j * TPC * R : (j + 1) * TPC * R],
                )
                for ko in range(j * TPC, (j + 1) * TPC):
                    nc.tensor.matmul(
                        gp2[:], ab16[:, ko, :], ab16[:, ko, :],
                        start=(ko == 0), stop=(ko == KO - 1),
                    )
            nc.any.tensor_copy(
                out=anat[:].rearrange("p t r -> p (t r)"), in_=ap1[:]
            )
            nc.scalar.dma_start(ou_d[:], anat[:])

            # ================= phase 2: v update (local N-shard) =================
            # c2 accumulation is emitted BEFORE the solve chain: the PE queue
            # is in-order, and the chain's smm matmuls wait on vector results;
            # queuing them first would block c2 behind those latencies. The
            # chain's vector/gpsimd prefix runs concurrently with c2 anyway.
            ct2 = bps.tile([R, SH], F32, tag="bps")
            KQ2 = KO // XQ
            for q in range(XQ):
                for ko in range(q * KQ2, (q + 1) * KQ2):
                    nc.tensor.matmul(
                        ct2[:], ab16[:, ko, :], xn[:, ko, :],
                        start=(ko == 0), stop=False,
                    )
            rd2 = small_chain(1, gp2)
            nc.tensor.matmul(
                ct2[:], sun16[:, 1, :], vt[:], start=False, stop=True
            )
            nc.vector.tensor_scalar(
                out=yb2[0:R, :], in0=ct2[:], scalar1=float(EPS), scalar2=rd2,
                op0=ALU.add, op1=ALU.mult,
            )
            ap2 = aps.tile([P, TS * R], F32, tag="aps2")
            for t in range(TS):
                nc.tensor.matmul(
                    ap2[:, t * R : (t + 1) * R],
                    yb2[:, t * P : (t + 1) * P], wzb[:, 1, :],
                )
            nc.any.tensor_copy(
                out=vnat[:].rearrange("p t r -> p (t r)"), in_=ap2[:]
            )
            nc.scalar.dma_start(ov_d[:], vnat[:])

    nc.compile()
    return nc


def _prep_in_maps(x, u, v):
    bf = ml_dtypes.bfloat16
    per_batch = []
    for b in range(B):
        xb = np.asarray(x[b], np.float32)
        xT = np.ascontiguousarray(xb.T).astype(bf)          # [N, M]
        # [P, MC, KO, SHC]: xtf[p, c, ko, m'] = xT[ko*128+p, c*SHC+m']
        xtf = np.ascontiguousarray(
            xT.reshape(KO, P, MC, SHC).transpose(1, 2, 0, 3)
        )
        vfb = np.asarray(v[b], np.float32)
        vf = np.ascontiguousarray(
            vfb.reshape(KO, P, R).swapaxes(0, 1)
        )
        vbb = vf.astype(bf)
        ut = np.ascontiguousarray(np.asarray(u[b], np.float32).T).astype(bf)
        per_batch.append((xb, xtf, vf, vbb, ut, vfb))
    in_maps = []
    for c in range(NCORES):
        b, s = divmod(c, NS)
        xb, xtf, vf, vbb, ut, vfb = per_batch[b]
        sl = slice(s * SH, (s + 1) * SH)
        xs = np.ascontiguousarray(xb[:, sl]).astype(bf)
        xn = np.ascontiguousarray(xs.reshape(KO, P, SH).swapaxes(0, 1))
        vts = np.ascontiguousarray(vfb[sl].T).astype(bf)
        in_maps.append(
            {"xtf": xtf, "xn": xn, "vf": vf, "vb": vbb, "ut": ut, "vt": vts}
        )
    return in_maps


def run(x, u, v, trace=False, trace_cores=None):
    if "nc" not in _CACHE:
        _CACHE["nc"] = _build_nc()
    nc = _CACHE["nc"]
    in_maps = _prep_in_maps(x, u, v)
    kw = {}
    if trace_cores is not None:
        kw["trace_cores"] = trace_cores
    res = run_bass_kernel_spmd(
        nc, in_maps, core_ids=list(range(NCORES)), trace=trace, **kw
    )
    u_new = np.empty((B, M, R), np.float32)
    v_new = np.empty((B, M, R), np.float32)
    for b in range(B):
        u_new[b] = (
            np.asarray(res.results[b * NS]["ou"]).transpose(1, 0, 2).reshape(M, R)
        )
    for c in range(NCORES):
        b, s = divmod(c, NS)
        sl = slice(s * SH, (s + 1) * SH)
        v_new[b, sl] = (
            np.asarray(res.results[c]["ov"]).transpose(1, 0, 2).reshape(SH, R)
        )
    return (u_new, v_new), res


def kernel(x, u, v):
    (u_new, v_new), _ = run(x, u, v, trace=bool(os.environ.get("CD_TRACE")))
    return (u_new, v_new)


# revision 12
# speedup vs baseline: 1.0793x; 1.0793x over previous
"""Trainium2 Bass kernel for nn_CoordinateDescent (B=2, M=N=2048, R=16).

Math: the coordinate-descent residual e never needs materializing. With
G = v^T v and c = x @ v, the per-rank recurrence collapses to a 16x16
triangular solve per row, computed via the nilpotent expansion
(I+Z)^-1 = (I-Z)(I+Z^2)(I+Z^4)(I+Z^8) -- log-depth small matmuls.

Sharding: 8 cores = batch (2) x N-shard (4). Every core computes the FULL
u_new for its batch from the full x^T (redundant 4x within a batch-group
but collective-free: measured ncfw AllGather here costs 60-80us dead time,
far more than the extra DMA). Phase 2 updates the core's own v N-shard.

Perf notes (v3): x^T is packed chunk-major in DRAM ([P, MC, KO, SHC]) so
every DMA descriptor is an 8KB contiguous run on both DRAM and SBUF
sides (1KB-run descriptors run at lower per-engine rates). xtf streams
in 4 chunks of 512 m-columns -- 512-wide matmul rhs amortizes the
~50ns/instruction PE overhead that 256-wide chunks double. The per-chunk
a-tile matmuls (ap1), bf16 cast, and phase-2 Gram partials all ride the
DMA-paced stream instead of the post-stream tail; xn follows xtf on the
same sync ring (FIFO keeps it from stealing stream bandwidth), quartered
so phase-2's c2 accumulation starts as soon as its slices land. The
phase-2 solve chain runs on vector/gpsimd/scalar under the xn stream.
Remote-DMA u-chunk exchange (which would cut the 4x x^T redundancy) was
prototyped and works functionally, but blocked waits on remotely-written
semaphores take 2-11ms in this runtime -- unusable.
"""

import os
import numpy as np
import ml_dtypes

import concourse.bass as bass
import concourse.mybir as mybir
import concourse.tile as tile
from concourse import bacc
from concourse.bass_utils import run_bass_kernel_spmd
from concourse.masks import make_identity

B, M, N, R = 2, 2048, 2048, 16
NCORES = 8
NS = 4            # N-shards per batch (cores per batch group)
SH = M // NS      # 512 (n-shard width)
P = 128
KO = M // P       # 16 k-tiles of 128
TS = SH // P      # 4 sub-tiles per shard
MC = 4            # m-chunks for the phase-1 cT stream
SHC = M // MC     # 512 m per chunk
XQ = 4            # xn stream quarters
EPS = 1e-8

F32 = mybir.dt.float32
BF16 = mybir.dt.bfloat16
ALU = mybir.AluOpType

_CACHE = {}


def _build_nc():
    nc = bacc.Bacc(
        "TRN2",
        target_bir_lowering=False,
        debug=False,
        num_devices=NCORES,
    )

    # x[b]^T chunk-major: [P, MC, KO, SHC]; per (p, chunk): 8KB contiguous
    xtf_d = nc.dram_tensor("xtf", [P, MC, KO, SHC], BF16, kind="ExternalInput")
    # x[b,:,nS] m-tiled: [P, KO, SH]; per p: 16KB contiguous
    xn_d = nc.dram_tensor("xn", [P, KO, SH], BF16, kind="ExternalInput")
    vf_d = nc.dram_tensor("vf", [P, KO, R], F32, kind="ExternalInput")     # v[b] tiled f32
    vb_d = nc.dram_tensor("vb", [P, KO, R], BF16, kind="ExternalInput")    # v[b] tiled bf16
    ut_d = nc.dram_tensor("ut", [R, M], BF16, kind="ExternalInput")        # full u[b]^T
    vt_d = nc.dram_tensor("vt", [R, SH], BF16, kind="ExternalInput")       # v[b,nS,:]^T
    ou_d = nc.dram_tensor("ou", [P, KO, R], F32, kind="ExternalOutput")    # full u_new
    ov_d = nc.dram_tensor("ov", [P, TS, R], F32, kind="ExternalOutput")    # v_new shard

    with tile.TileContext(nc, num_cores=NCORES) as tc:
        with (
            tc.tile_pool(name="big", bufs=1) as big,
            tc.tile_pool(name="cst", bufs=1) as cst,
            tc.tile_pool(name="ya", bufs=1) as ya,
            tc.tile_pool(name="gps", bufs=1, space="PSUM") as gps,
            tc.tile_pool(name="sps", bufs=2, space="PSUM") as sps,
            tc.tile_pool(name="bps", bufs=2, space="PSUM") as bps,
            tc.tile_pool(name="aps", bufs=1, space="PSUM") as aps,
        ):
            vf = cst.tile([P, KO, R], F32, tag="vf")
            vb = cst.tile([P, KO, R], BF16, tag="vb")
            utf = cst.tile([P, M], BF16, tag="utf")     # rows 16+ zero
            vt = cst.tile([P, SH], BF16, tag="vt")      # rows 16+ zero
            ident = cst.tile([P, R], F32, tag="ident")  # I16 in rows 0:16
            misc = cst.tile([P, 8], F32, tag="misc")    # d / rd columns
            anat = cst.tile([P, KO, R], F32, tag="anat")
            ab16 = cst.tile([P, KO, R], BF16, tag="ab16")
            vnat = cst.tile([P, TS, R], F32, tag="vnat")

            NSLOT = 18
            arena = cst.tile([P, 2 * NSLOT, R], F32, tag="arena")
            sun16 = cst.tile([P, 2, R], BF16, tag="sun16")
            wzb = cst.tile([P, 2, R], BF16, tag="wzb")
            yb1 = ya.tile([P, M], BF16, tag="yb1")      # rows 16+ zero
            yb2 = ya.tile([P, SH], BF16, tag="yb2")

            nc.any.memzero(utf[:])
            nc.any.memzero(vt[:])
            nc.any.memzero(ident[:])
            nc.any.memzero(arena[:])
            nc.any.memzero(sun16[:])
            nc.any.memzero(wzb[:])
            nc.any.memzero(yb1[:])
            nc.any.memzero(yb2[:])
            make_identity(nc, ident[0:R, 0:R], nomemset=True)

            # Small inputs lead the sync ring; the big streams follow FIFO.
            nc.sync.dma_start(vf[:], vf_d[:])
            nc.sync.dma_start(vb[:], vb_d[:])
            nc.sync.dma_start(utf[0:R, :], ut_d[:])
            nc.sync.dma_start(vt[0:R, :], vt_d[:])

            xtf = big.tile([P, MC, KO, SHC], BF16, tag="xtf")
            xn = big.tile([P, KO, SH], BF16, tag="xn")
            # ko-sliced chunk DMAs (8KB/partition runs) so the c-matmuls
            # start as soon as the first k-tiles land, not per 2MB chunk;
            # the kernel is PE-bound end to end, so an earlier PE start
            # shifts the whole pipeline left.
            for j in range(MC):
                kq = 4 if j == 0 else 8
                for q in range(KO // kq):
                    s = slice(q * kq, (q + 1) * kq)
                    nc.sync.dma_start(xtf[:, j, s, :], xtf_d[:, j, s, :])
            KQ = KO // XQ
            for q in range(XQ):
                s = slice(q * KQ, (q + 1) * KQ)
                nc.sync.dma_start(xn[:, s, :], xn_d[:, s, :])

            def slot(ph, i):
                return arena[:, ph * NSLOT + i, :]

            def slot16(ph, i):
                return arena[0:R, ph * NSLOT + i, :]

            def smm(out_slot16, lhsT_pad, rhs_pad):
                ps = sps.tile([R, R], F32, tag="sps")
                nc.tensor.matmul(ps[:], lhsT_pad, rhs_pad)
                nc.any.tensor_copy(out=out_slot16, in_=ps[:])

            I16 = ident[0:R, 0:R]

            def small_chain(ph, g_psum):
                G = slot16(ph, 0)
                nc.any.tensor_copy(out=G, in_=g_psum[:])
                d = misc[0:R, 4 * ph + 0 : 4 * ph + 1]
                rd = misc[0:R, 4 * ph + 1 : 4 * ph + 2]
                gd = slot16(ph, 1)
                nc.vector.tensor_tensor(gd, G, I16, ALU.mult)
                nc.vector.tensor_reduce(d, gd, axis=mybir.AxisListType.X, op=ALU.add)
                nc.vector.tensor_scalar_add(d, d, float(EPS))
                nc.vector.reciprocal(rd, d)
                nc.any.tensor_scalar_mul(gd, G, -1.0)
                slnf = slot16(ph, 15)
                nc.gpsimd.affine_select(
                    out=slnf, in_=gd, compare_op=ALU.is_ge, fill=0.0,
                    base=-1, pattern=[[-1, R]], channel_multiplier=1,
                )
                nc.any.tensor_copy(out=sun16[0:R, ph, :], in_=slnf)
                SL = slot16(ph, 2)
                nc.gpsimd.affine_select(
                    out=SL, in_=G, compare_op=ALU.is_ge, fill=0.0,
                    base=-1, pattern=[[-1, R]], channel_multiplier=1,
                )
                Z = slot16(ph, 3)
                nc.vector.tensor_scalar_mul(Z, SL, rd)
                smm(slot16(ph, 4), slot(ph, 3), ident[:, 0:R])  # zt1 = Z^T
                smm(slot16(ph, 5), slot(ph, 4), slot(ph, 3))   # z2
                smm(slot16(ph, 6), slot(ph, 3), slot(ph, 4))   # zt2
                smm(slot16(ph, 7), slot(ph, 6), slot(ph, 5))   # z4
                smm(slot16(ph, 8), slot(ph, 5), slot(ph, 6))   # zt4
                smm(slot16(ph, 9), slot(ph, 8), slot(ph, 7))   # z8
                nc.vector.tensor_tensor(slot16(ph, 10), I16, slot16(ph, 4), ALU.subtract)
                nc.vector.tensor_tensor(slot16(ph, 11), I16, slot16(ph, 5), ALU.add)
                nc.vector.tensor_tensor(slot16(ph, 12), I16, slot16(ph, 8), ALU.add)
                nc.vector.tensor_tensor(slot16(ph, 13), I16, slot16(ph, 9), ALU.add)
                smm(slot16(ph, 14), slot(ph, 11), slot(ph, 10))  # P1
                smm(slot16(ph, 15), slot(ph, 12), slot(ph, 13))  # o2T
                smm(slot16(ph, 16), slot(ph, 15), slot(ph, 14))  # WzT
                nc.any.tensor_copy(out=wzb[0:R, ph, :], in_=slot16(ph, 16))
                return rd

            # ================= phase 1: full u update =================
            gp = gps.tile([R, R], F32, tag="gps")
            for ko in range(KO):
                nc.tensor.matmul(
                    gp[:], vf[:, ko, :], vf[:, ko, :],
                    start=(ko == 0), stop=(ko == KO - 1),
                )
            TPC = KO // MC   # ap1/G2 tiles finished per chunk
            rd1 = None
            ap1 = aps.tile([P, KO * R], F32, tag="aps1")
            gp2 = gps.tile([R, R], F32, tag="gps2")
            for j in range(MC):
                ms = slice(j * SHC, (j + 1) * SHC)
                ct = bps.tile([R, SHC], F32, tag="bps")
                for ko in range(KO):
                    nc.tensor.matmul(
                        ct[:], vb[:, ko, :], xtf[:, j, ko, :],
                        start=(ko == 0), stop=False,
                    )
                if j == 0:
                    rd1 = small_chain(0, gp)   # hides in the DMA-paced stream
                nc.tensor.matmul(
                    ct[:], sun16[:, 0, :], utf[:, ms], start=False, stop=True
                )
                nc.vector.tensor_scalar(
                    out=yb1[0:R, ms], in0=ct[:], scalar1=float(EPS), scalar2=rd1,
                    op0=ALU.add, op1=ALU.mult,
                )
                # ap1 tiles, bf16 cast, and G2 partials for this chunk ride
                # the DMA-paced stream instead of the post-stream tail
                for t in range(j * TPC, (j + 1) * TPC):
                    nc.tensor.matmul(
                        ap1[:, t * R : (t + 1) * R],
                        yb1[:, t * P : (t + 1) * P], wzb[:, 0, :],
                    )
                nc.vector.tensor_copy(
                    out=ab16[:, j * TPC : (j + 1) * TPC, :].rearrange(
                        "p t r -> p (t r)"
                    ),
                    in_=ap1[:, j * TPC * R : (j + 1) * TPC * R],
                )
                for ko in range(j * TPC, (j + 1) * TPC):
                    nc.tensor.matmul(
                        gp2[:], ab16[:, ko, :], ab16[:, ko, :],
                        start=(ko == 0), stop=(ko == KO - 1),
                    )
            nc.any.tensor_copy(
                out=anat[:].rearrange("p t r -> p (t r)"), in_=ap1[:]
            )
            nc.scalar.dma_start(ou_d[:], anat[:])

            # ================= phase 2: v update (local N-shard) =================
            rd2 = small_chain(1, gp2)
            ct2 = bps.tile([R, SH], F32, tag="bps")
            KQ2 = KO // XQ
            for q in range(XQ):
                for ko in range(q * KQ2, (q + 1) * KQ2):
                    nc.tensor.matmul(
                        ct2[:], ab16[:, ko, :], xn[:, ko, :],
                        start=(ko == 0), stop=False,
                    )
            nc.tensor.matmul(
                ct2[:], sun16[:, 1, :], vt[:], start=False, stop=True
            )
            nc.vector.tensor_scalar(
                out=yb2[0:R, :], in0=ct2[:], scalar1=float(EPS), scalar2=rd2,
                op0=ALU.add, op1=ALU.mult,
            )
            ap2 = aps.tile([P, TS * R], F32, tag="aps2")
            for t in range(TS):
                nc.tensor.matmul(
                    ap2[:, t * R : (t + 1) * R],
                    yb2[:, t * P : (t + 1) * P], wzb[:, 1, :],
                )
            nc.any.tensor_copy(
                out=vnat[:].rearrange("p t r -> p (t r)"), in_=ap2[:]
            )
            nc.scalar.dma_start(ov_d[:], vnat[:])

    nc.compile()
    return nc


def _prep_in_maps(x, u, v):
    bf = ml_dtypes.bfloat16
    per_batch = []
    for b in range(B):
        xb = np.asarray(x[b], np.float32)
        xT = np.ascontiguousarray(xb.T).astype(bf)          # [N, M]
        # [P, MC, KO, SHC]: xtf[p, c, ko, m'] = xT[ko*128+p, c*SHC+m']
        xtf = np.ascontiguousarray(
            xT.reshape(KO, P, MC, SHC).transpose(1, 2, 0, 3)
        )
        vfb = np.asarray(v[b], np.float32)
        vf = np.ascontiguousarray(
            vfb.reshape(KO, P, R).swapaxes(0, 1)
        )
        vbb = vf.astype(bf)
        ut = np.ascontiguousarray(np.asarray(u[b], np.float32).T).astype(bf)
        per_batch.append((xb, xtf, vf, vbb, ut, vfb))
    in_maps = []
    for c in range(NCORES):
        b, s = divmod(c, NS)
        xb, xtf, vf, vbb, ut, vfb = per_batch[b]
        sl = slice(s * SH, (s + 1) * SH)
        xs = np.ascontiguousarray(xb[:, sl]).astype(bf)
        xn = np.ascontiguousarray(xs.reshape(KO, P, SH).swapaxes(0, 1))
        vts = np.ascontiguousarray(vfb[sl].T).astype(bf)
        in_maps.append(
            {"xtf": xtf, "xn": xn, "vf": vf, "vb": vbb, "ut": ut, "vt": vts}
        )
    return in_maps


def run(x, u, v, trace=False, trace_cores=None):
    if "nc" not in _CACHE:
        _CACHE["nc"] = _build_nc()
    nc = _CACHE["nc"]
    in_maps = _prep_in_maps(x, u, v)
    kw = {}
    if trace_cores is not None:
        kw["trace_cores"] = trace_cores
    res = run_bass_kernel_spmd(
        nc, in_maps, core_ids=list(range(NCORES)), trace=trace, **kw
    )
    u_new = np.empty((B, M, R), np.float32)
    v_new = np.empty((B, M, R), np.float32)
    for b in range(B):
        u_new[b] = (
            np.asarray(res.results[b * NS]["ou"]).transpose(1, 0, 2).reshape(M, R)
        )
    for c in range(NCORES):
        b, s = divmod(c, NS)
        sl = slice(s * SH, (s + 1) * SH)
        v_new[b, sl] = (
            np.asarray(res.results[c]["ov"]).transpose(1, 0, 2).reshape(SH, R)
        )
    return (u_new, v_new), res


def kernel(x, u, v):
    (u_new, v_new), _ = run(x, u, v, trace=bool(os.environ.get("CD_TRACE")))
    return (u_new, v_new)
